# revision 44
# baseline (speedup 1.0000x reference)
"""GCNConv-local Trainium2 kernel (8 NeuronCores, SPMD).

Math (reference):
    deg_i = 1 + #valid(edge_index[i]);  isd = deg^-0.5
    h = (x @ W.T) * isd
    out_i = (sum_d h[e_id] + h_i) * isd_i

Reformulated so the 256-wide matmul happens AFTER the neighbor sum, on only
the local shard (weight application commutes with the row-sum):
    xs_j  = isd_j * x_j                      (full table, built per core)
    y_i   = xs_i + sum_d xs[e_id]            (gather + slot accumulation)
    out_i = isd_i * (y_i @ W.T)

Sharding: nodes split contiguously across the 8 cores; the scaled table is
replicated so no collectives are needed.

The gather is bound by the Pool engine's SWDGE descriptor-generation ucode
(~1.1us per indirect DMA, 128 rows each; the indirect1d ucode supports only
one offset per partition per instruction). To minimize and hide that cost:
  - host pre-pass sorts each node's 17 slots ascending (sentinels compact to
    the back and truncate), sorts nodes by degree so each 128-row tile needs
    only max-degree-in-tile gather slots (~26% fewer instructions), and
    un-permutes the rows on the way out;
  - the table is built in global-row-prefix order and each gather declares a
    shrunken source extent + explicit dep on the covering build slab, so
    early slot rounds (small sorted indices) start while the build is still
    streaming;
  - gathers land in a small ring and are folded into per-tile bf16
    accumulators; the transpose+matmul tail runs as soon as a tile's last
    slot round completes.
"""

import sys

if "/opt/trn_rl_repo" not in sys.path:
    sys.path.insert(0, "/opt/trn_rl_repo")

import numpy as np

import concourse.bass as bass
import concourse.mybir as mybir
from concourse.bass import IndirectOffsetOnAxis
from concourse.masks import make_identity
from concourse.tile import TileContext, add_dep_helper

P = 128
D = 256
MAXD = 16
MAXS = 17  # slots per node: 16 neighbors + self (sorted, sentinel-compacted)
NCORES = 8
SLAB = 1024  # rows per build slab (prefix order)

F32 = mybir.dt.float32
BF16 = mybir.dt.bfloat16
I32 = mybir.dt.int32

XS_DT = BF16  # gathered-table dtype (bf16 halves build-write + gather traffic)


# ---------------------------------------------------------------------------
# walrus workaround: this image's walrus rejects >1-2 sync waits on one
# instruction. Split the Tile tail-drain across single-wait NOPs and hoist
# excess waits from every instruction onto preceding same-engine NOPs.
# ---------------------------------------------------------------------------

def _install_tile_fix():
    import bass_rust
    import concourse.tile as tile_mod
    from concourse.tile import TileContext as TC

    def _split_drain_and_barrier(self, tick_clock, wait_clock):
        gc = tick_clock.global_clock
        for i, t in enumerate(list(gc)):
            if t > 0:
                vc_l = [0] * len(list(gc))
                vc_l[i] = t
                nop = self.nc.sync.nop(nofuse=True, hint=f"drain_wait_{i}")
                wait_clock.add_sem_waits(
                    nop.ins,
                    tile_mod.ScopedClock({None: bass_rust.VectorClock(vc_l)}),
                )
        self.nc.sync.drain()
        self.nc.all_engine_barrier()
        assert self.sems is not None
        popped = self.nc._tile_sem_poison_stack.pop()
        assert popped is self._sem_poison
        self.nc.clear_and_free_semaphores(list(self.sems.allocated().values()))
        self.nc.all_engine_barrier()

    TC._drain_and_barrier = _split_drain_and_barrier


_install_tile_fix()

_MAX_WAITS = 1


def _fix_sync_waits(nc):
    n_fixed = 0
    for fn in nc.m.functions:
        for bb in fn.blocks:
            new_insts = []
            for inst in bb.instructions:
                si = inst.sync_info
                if si is not None and si.on_wait and len(si.on_wait) > _MAX_WAITS:
                    waits = list(si.on_wait)
                    keep = waits[-_MAX_WAITS:]
                    extra = waits[:-_MAX_WAITS]
                    for i in range(0, len(extra), _MAX_WAITS):
                        chunk = extra[i : i + _MAX_WAITS]
                        nop = mybir.InstNoOp(
                            name=nc.get_next_instruction_name(),
                            engine=inst.engine,
                            ins=[],
                            outs=[],
                            sync_info=mybir.SyncInfo(on_wait=chunk, on_update=[]),
                            bass_nofuse=True,
                            text_hint="split_wait",
                        )
                        nc.register_instruction(nop)
                        new_insts.append(nop)
                    si.on_wait = keep
                    n_fixed += 1
                new_insts.append(inst)
            bb.instructions[:] = new_insts
    return n_fixed


# ---------------------------------------------------------------------------
# kernel builder (one SPMD module; per-core data arrives via in_maps)
# ---------------------------------------------------------------------------

def build_nc(npad, k_tiles, col_max):
    """npad: padded node count (multiple of 128*NCORES).
    k_tiles[t]: gather slots needed for tile t (max across cores).
    col_max[t][s]: max table row touched by column (t, s) across cores
                   (0 = sentinel/zero row)."""
    nl = npad // NCORES          # nodes per core
    t_shard = nl // P            # shard tiles per core
    sup = SLAB // P              # rows-of-128 per build slab
    n_slab = npad // SLAB
    ct = npad // P // 8          # isd chunk -> 8 chunks

    nc = bass.Bass("TRN2")
    x = nc.dram_tensor("x", [npad, D], F32, kind="ExternalInput")
    # host-prelayouted index tensors (partition-major, contiguous per
    # partition so loads are few fat descriptors):
    #   gidx[p, g*sup+r, s]  = slots of node g*SLAB + p*sup + r
    #   sgidx[p, t*MAXS + s] = sorted slot s of shard node t*128 + p
    gidx = nc.dram_tensor(
        "gidx", [P, (npad // P) * MAXS], I32, kind="ExternalInput"
    )
    sgidx = nc.dram_tensor(
        "sgidx", [P, (nl // P) * MAXS], I32, kind="ExternalInput"
    )
    wt = nc.dram_tensor("wt", [D, D], F32, kind="ExternalInput")
    out = nc.dram_tensor("out", [nl, D], F32, kind="ExternalOutput")
    # row 0 = zero row (sentinel target); node n lives at row n+1
    xs = nc.dram_tensor("xs", [npad + P, D], XS_DT)

    with TileContext(nc) as tc:
        with (
            tc.tile_pool(name="const", bufs=1) as cpool,
            tc.tile_pool(name="deg", bufs=2) as dpool,
            tc.tile_pool(name="build", bufs=5) as bpool,
            tc.tile_pool(name="accp", bufs=1) as apool,
            tc.tile_pool(name="ring", bufs=12) as rpool,
            tc.tile_pool(name="tail", bufs=4) as tpool,
            tc.tile_pool(name="psum", bufs=4, space="PSUM") as ppool,
        ):
            # --- constants -------------------------------------------------
            ident = cpool.tile([P, P], BF16, name="ident")
            make_identity(nc, ident[:])

            wtf = cpool.tile([P, 2, D], F32, name="wtf")
            nc.sync.dma_start(wtf[:], wt[:].rearrange("(c p) d -> p c d", p=P))
            wtb = cpool.tile([P, 2, D], BF16, name="wtb")
            nc.vector.tensor_copy(wtb[:], wtf[:])

            # zero row (sentinel target) -- written before any build slab
            zt = bpool.tile([P, D], XS_DT, name="zt")
            nc.vector.memset(zt[:], 0.0)
            zrow = nc.sync.dma_start(xs[0:P, :], zt[:])

            # sorted shard indices loaded FIRST, in ONE DMA (a single
            # completion-sem value lets the wait on it be elided along the
            # pool-ordered gather chain)
            sg = cpool.tile([P, t_shard, MAXS], I32, name="sg")
            nc.sync.dma_start(
                sg[:], sgidx[:].rearrange("p (t s) -> p t s", s=MAXS)
            )

            # --- full-table isd [P, rows_pp], node g*SLAB + p*sup + r ------
            rows_pp = npad // P
            isd = cpool.tile([P, rows_pp], F32, name="isd")
            gv = gidx[:].rearrange("p (c s) -> p c s", s=MAXS)
            for c0 in range(0, rows_pp, ct):
                gt = dpool.tile([P, ct, MAXS], I32, name="gt")
                nc.sync.dma_start(gt[:], gv[:, c0 : c0 + ct, :])
                m = dpool.tile([P, ct, MAXS], F32, name="m")
                nc.vector.tensor_scalar(
                    m[:], gt[:], npad - 1, None, op0=mybir.AluOpType.is_le
                )
                dg = dpool.tile([P, ct], F32, name="dg")
                nc.vector.reduce_sum(dg[:], m[:], axis=mybir.AxisListType.X)
                nc.scalar.activation(
                    dg[:], dg[:], mybir.ActivationFunctionType.Sqrt
                )
                nc.vector.reciprocal(isd[:, c0 : c0 + ct], dg[:])

            # --- shard isd [P, t_shard] from sorted shard indices ----------
            # sgidx values: 0 = sentinel (zero row), else table row (node+1)
            isd_sh = cpool.tile([P, t_shard], F32, name="isd_sh")
            msh = dpool.tile([P, t_shard, MAXS], F32, name="msh")
            nc.vector.tensor_scalar(
                msh[:], sg[:], 1, None, op0=mybir.AluOpType.is_ge
            )
            dgs = dpool.tile([P, t_shard], F32, name="dgs")
            nc.vector.reduce_sum(dgs[:], msh[:], axis=mybir.AxisListType.X)
            nc.scalar.activation(
                dgs[:], dgs[:], mybir.ActivationFunctionType.Sqrt
            )
            nc.vector.reciprocal(isd_sh[:], dgs[:])

            # --- phase 1: xs[n+1] = x[n] * isd[n], global prefix order -----
            # slab g covers nodes [g*SLAB, (g+1)*SLAB) -> rows +1; inside a
            # slab, partition p owns sup CONTIGUOUS rows (fat descriptors)
            BPRI = -20_000_000  # build streams ahead of everything
            slab_store = []
            for g in range(n_slab):
                xt = bpool.tile([P, sup, D], F32, name="xt")
                ld = nc.sync.dma_start(
                    xt[:],
                    x[g * SLAB : (g + 1) * SLAB, :].rearrange(
                        "(p r) d -> p r d", p=P
                    ),
                )
                ld.ins.bass_priority = BPRI + 16 * g
                xst = bpool.tile([P, sup, D], XS_DT, name="xst")
                for r in range(sup):
                    # alternate scale engines so DVE and Act stream slabs in
                    # parallel (DVE is idle until the gather adds ramp up)
                    if g % 2 == 0:
                        sc = nc.vector.tensor_scalar_mul(
                            xst[:, r, :], xt[:, r, :],
                            isd[:, g * sup + r : g * sup + r + 1],
                        )
                    else:
                        sc = nc.scalar.mul(
                            xst[:, r, :], xt[:, r, :],
                            isd[:, g * sup + r : g * sup + r + 1],
                        )
                    sc.ins.bass_priority = BPRI + 16 * g + 1 + r
                st = nc.sync.dma_start(
                    xs[1 + g * SLAB : 1 + (g + 1) * SLAB, :].rearrange(
                        "(p r) d -> p r d", p=P
                    ),
                    xst[:],
                )
                st.ins.bass_priority = BPRI + 16 * g + 15
                slab_store.append(st)

            # --- phase 2: canonical gather stream + accumulate -------------
            # src extent is declared tiny (deps via explicit slab waits only;
            # descriptors address the full tensor regardless). Columns run in
            # the SAME s-major order the host used to build the first-use
            # table permutation, so col_max values are monotone along the
            # stream. Pool executes in (nosync-pinned) order, so one wait per
            # slab TRANSITION orders everything after it.
            # k_tiles is non-increasing, so each round's alive tiles are the
            # prefix [0, cnt). 4 consecutive tiles share one ring pack and
            # ONE wide DVE accumulate into a contiguous acc slab -> the next
            # ring cycle's 4 gathers share a single prior reader and all but
            # the first WAR wait elide.
            max_k = max(k_tiles)
            accbig = apool.tile([P, t_shard, D], XS_DT, name="accbig")
            prev_gather = None
            cur_slab = -1  # last slab the Pool stream is ordered after
            GPACK = 4      # consecutive-tile columns sharing one ring tile
            PRI0 = -10_000_000  # schedule the gather stream ahead of build
            pri = 0
            for s in range(max_k):
                cnt = sum(1 for kk in k_tiles if kk > s)
                for g0 in range(0, cnt, GPACK):
                    gw = min(GPACK, cnt - g0)
                    gb = rpool.tile([P, GPACK, D], XS_DT, name="gb")
                    for j in range(gw):
                        t = g0 + j
                        mv = col_max[t][s]
                        gsl = -1 if mv <= 0 else (mv - 1) // SLAB
                        bi = nc.gpsimd.indirect_dma_start(
                            out=gb[:, j, :],
                            out_offset=None,
                            in_=xs[0:P, :],
                            in_offset=IndirectOffsetOnAxis(
                                ap=sg[:, t, s : s + 1], axis=0
                            ),
                            compute_op=mybir.AluOpType.bypass,
                        )
                        bi.ins.bass_priority = PRI0 + pri
                        pri += 1
                        if prev_gather is None:
                            add_dep_helper(
                                bi.ins, zrow.ins, reason="zero row"
                            )
                        else:
                            # ordering-only: pins the scheduler's Pool order
                            # so slab-transition waits cover later gathers
                            add_dep_helper(
                                bi.ins, prev_gather.ins, sync=False,
                                reason="pool order",
                            )
                        prev_gather = bi
                        if gsl > cur_slab:
                            add_dep_helper(
                                bi.ins, slab_store[gsl].ins,
                                reason="xs prefix built",
                            )
                            cur_slab = gsl
                    dst = accbig[:, g0 : g0 + gw, :]
                    if s == 0:
                        vi = nc.vector.tensor_copy(dst, gb[:, 0:gw, :])
                    else:
                        vi = nc.vector.tensor_add(dst, dst, gb[:, 0:gw, :])
                    vi.ins.bass_priority = PRI0 + pri
                    pri += 1
                # tails for tiles whose last round just completed
                for t in range(t_shard):
                    if k_tiles[t] != s + 1:
                        continue
                    ytt = tpool.tile([P, 2, P], BF16, name="ytt")
                    for ci in range(2):
                        pt = ppool.tile([P, P], BF16, name="pt")
                        nc.tensor.transpose(
                            pt[:],
                            accbig[:, t, ci * P : (ci + 1) * P],
                            ident[:],
                        )
                        nc.vector.tensor_copy(ytt[:, ci, :], pt[:])
                    po = ppool.tile([P, D], F32, name="po")
                    for ci in range(2):
                        nc.tensor.matmul(
                            po[:],
                            ytt[:, ci, :],
                            wtb[:, ci, :],
                            start=(ci == 0),
                            stop=(ci == 1),
                        )
                    ot = tpool.tile([P, D], F32, name="ot")
                    nc.vector.tensor_scalar_mul(
                        ot[:], po[:], isd_sh[:, t : t + 1]
                    )
                    nc.scalar.dma_start(
                        out[t * P : (t + 1) * P, :], ot[:]
                    )

    _fix_sync_waits(nc)
    return nc


# ---------------------------------------------------------------------------
# host entry point
# ---------------------------------------------------------------------------

def _prep(x, edge_index, W):
    x = np.ascontiguousarray(np.asarray(x, dtype=np.float32))
    ei = np.asarray(edge_index)
    W = np.ascontiguousarray(np.asarray(W, dtype=np.float32))
    n = x.shape[0]
    npad = -(-n // (P * NCORES)) * (P * NCORES)
    nl = npad // NCORES
    t_shard = nl // P

    xp = np.zeros((npad, D), np.float32)
    xp[:n] = x

    # full-table slot lists (UNSORTED; sentinel npad; self in last slot) --
    # only used on-device to compute isd for the build scaling
    gi = np.full((npad, MAXS), npad, np.int32)
    gi[:, MAXS - 1] = np.arange(npad, dtype=np.int32)
    e = ei.astype(np.int64)
    gi[:n, : MAXS - 1] = np.where(e < 0, npad, e).astype(np.int32)
    wt = np.ascontiguousarray(W.T)

    # per-core shard prep: degree sort + sentinel-compacting slot sort -----
    deg_all = (gi < npad).sum(axis=1)  # includes self
    orders, shss = [], []
    kc = np.zeros((NCORES, t_shard), np.int32)
    for c in range(NCORES):
        sh = gi[c * nl : (c + 1) * nl]
        dg = deg_all[c * nl : (c + 1) * nl]
        order = np.argsort(-dg, kind="stable")
        shs = np.sort(sh[order], axis=1)          # ascending; sentinels last
        orders.append(order)
        shss.append(shs)
        kc[c] = dg[order].reshape(t_shard, P).max(axis=1)
    k_tiles = kc.max(axis=0).tolist()
    max_k = max(k_tiles)

    # demand-driven table permutation: walk the canonical column stream
    # (s-major rounds, all cores merged) and assign table slots in FIRST-USE
    # order. Every column then needs only the build prefix up to its own
    # position -> the build never blocks the gather stream.
    pi = np.full(npad, -1, np.int64)         # node -> 0-based table slot
    nassign = 0
    col_max = [[0] * MAXS for _ in range(t_shard)]
    shs3 = [s.reshape(t_shard, P, MAXS) for s in shss]
    for s in range(max_k):                   # s-major: matches emission
        for t in range(t_shard):
            if k_tiles[t] <= s:
                continue
            for c in range(NCORES):
                vals = shs3[c][t, :, s]
                real = vals[vals < npad]
                new = real[pi[real] < 0]
                if new.size:
                    new = np.unique(new)
                    pi[new] = np.arange(
                        nassign, nassign + new.size, dtype=np.int64
                    )
                    nassign += new.size
            col_max[t][s] = nassign  # table rows 1..nassign cover this col
    left = np.where(pi < 0)[0]
    pi[left] = np.arange(nassign, nassign + left.size, dtype=np.int64)

    sgidxs = []
    for c in range(NCORES):
        shs = shss[c]
        vals = np.where(
            shs >= npad, 0, pi[np.minimum(shs, npad - 1)] + 1
        ).astype(np.int32)
        sgidxs.append(np.ascontiguousarray(vals))

    # permute node rows of x and gidx into table-slot order
    inv_rows = np.empty(npad, np.int64)
    inv_rows[pi] = np.arange(npad, dtype=np.int64)
    xp = xp[inv_rows]
    gi_perm = gi[inv_rows]

    # partition-major device layouts (fat DMA descriptors):
    # gidx_dev[p, (g*sup+r)*MAXS+s] = gi_perm[g*SLAB + p*sup + r, s]
    sup = SLAB // P
    n_slab = npad // SLAB
    gidx_dev = np.ascontiguousarray(
        gi_perm.reshape(n_slab, P, sup, MAXS)
        .transpose(1, 0, 2, 3)
        .reshape(P, -1)
    )
    t_shard_ = nl // P

    in_maps = []
    for c in range(NCORES):
        sg_dev = np.ascontiguousarray(
            sgidxs[c].reshape(t_shard_, P, MAXS).transpose(1, 0, 2).reshape(P, -1)
        )
        in_maps.append(
            {"x": xp, "gidx": gidx_dev, "sgidx": sg_dev, "wt": wt}
        )
    return npad, n, in_maps, orders, k_tiles, col_max


def kernel(x, edge_index, W, trace=False):
    from concourse.bass_utils import run_bass_kernel_spmd

    npad, n, in_maps, orders, k_tiles, col_max = _prep(x, edge_index, W)
    nl = npad // NCORES
    nc = build_nc(npad, k_tiles, col_max)
    res = run_bass_kernel_spmd(
        nc, in_maps, core_ids=list(range(NCORES)), trace=trace
    )
    out = np.empty((npad, D), np.float32)
    for c in range(NCORES):
        blk = out[c * nl : (c + 1) * nl]
        blk[orders[c]] = res.results[c]["out"]
    kernel.last_exec_time_ns = res.exec_time_ns
    kernel.last_results = res
    return out[:n].astype(np.float32)


kernel.last_exec_time_ns = None


# revision 45
# speedup vs baseline: 1.0151x; 1.0151x over previous
"""GCNConv-local Trainium2 kernel (8 NeuronCores, SPMD).

Math (reference):
    deg_i = 1 + #valid(edge_index[i]);  isd = deg^-0.5
    h = (x @ W.T) * isd
    out_i = (sum_d h[e_id] + h_i) * isd_i

Reformulated so the 256-wide matmul happens AFTER the neighbor sum, on only
the local shard (weight application commutes with the row-sum):
    xs_j  = isd_j * x_j                      (full table, built per core)
    y_i   = xs_i + sum_d xs[e_id]            (gather + slot accumulation)
    out_i = isd_i * (y_i @ W.T)

Sharding: nodes split contiguously across the 8 cores; the scaled table is
replicated so no collectives are needed.

The gather is bound by the Pool engine's SWDGE descriptor-generation ucode
(~1.1us per indirect DMA, 128 rows each; the indirect1d ucode supports only
one offset per partition per instruction). To minimize and hide that cost:
  - host pre-pass sorts each node's 17 slots ascending (sentinels compact to
    the back and truncate), sorts nodes by degree so each 128-row tile needs
    only max-degree-in-tile gather slots (~26% fewer instructions), and
    un-permutes the rows on the way out;
  - the table is built in global-row-prefix order and each gather declares a
    shrunken source extent + explicit dep on the covering build slab, so
    early slot rounds (small sorted indices) start while the build is still
    streaming;
  - gathers land in a small ring and are folded into per-tile bf16
    accumulators; the transpose+matmul tail runs as soon as a tile's last
    slot round completes.
"""

import sys

if "/opt/trn_rl_repo" not in sys.path:
    sys.path.insert(0, "/opt/trn_rl_repo")

import numpy as np

import concourse.bass as bass
import concourse.mybir as mybir
from concourse.bass import IndirectOffsetOnAxis
from concourse.masks import make_identity
from concourse.tile import TileContext, add_dep_helper

P = 128
D = 256
MAXD = 16
MAXS = 17  # slots per node: 16 neighbors + self (sorted, sentinel-compacted)
NCORES = 8
SLAB = 1024  # rows per build slab (prefix order)

F32 = mybir.dt.float32
BF16 = mybir.dt.bfloat16
I32 = mybir.dt.int32

XS_DT = BF16  # gathered-table dtype (bf16 halves build-write + gather traffic)


# ---------------------------------------------------------------------------
# walrus workaround: this image's walrus rejects >1-2 sync waits on one
# instruction. Split the Tile tail-drain across single-wait NOPs and hoist
# excess waits from every instruction onto preceding same-engine NOPs.
# ---------------------------------------------------------------------------

def _install_tile_fix():
    import bass_rust
    import concourse.tile as tile_mod
    from concourse.tile import TileContext as TC

    def _split_drain_and_barrier(self, tick_clock, wait_clock):
        gc = tick_clock.global_clock
        for i, t in enumerate(list(gc)):
            if t > 0:
                vc_l = [0] * len(list(gc))
                vc_l[i] = t
                nop = self.nc.sync.nop(nofuse=True, hint=f"drain_wait_{i}")
                wait_clock.add_sem_waits(
                    nop.ins,
                    tile_mod.ScopedClock({None: bass_rust.VectorClock(vc_l)}),
                )
        self.nc.sync.drain()
        self.nc.all_engine_barrier()
        assert self.sems is not None
        popped = self.nc._tile_sem_poison_stack.pop()
        assert popped is self._sem_poison
        self.nc.clear_and_free_semaphores(list(self.sems.allocated().values()))
        self.nc.all_engine_barrier()

    TC._drain_and_barrier = _split_drain_and_barrier


_install_tile_fix()

_MAX_WAITS = 1


def _fix_sync_waits(nc):
    n_fixed = 0
    for fn in nc.m.functions:
        for bb in fn.blocks:
            new_insts = []
            for inst in bb.instructions:
                si = inst.sync_info
                if si is not None and si.on_wait and len(si.on_wait) > _MAX_WAITS:
                    waits = list(si.on_wait)
                    keep = waits[-_MAX_WAITS:]
                    extra = waits[:-_MAX_WAITS]
                    for i in range(0, len(extra), _MAX_WAITS):
                        chunk = extra[i : i + _MAX_WAITS]
                        nop = mybir.InstNoOp(
                            name=nc.get_next_instruction_name(),
                            engine=inst.engine,
                            ins=[],
                            outs=[],
                            sync_info=mybir.SyncInfo(on_wait=chunk, on_update=[]),
                            bass_nofuse=True,
                            text_hint="split_wait",
                        )
                        nc.register_instruction(nop)
                        new_insts.append(nop)
                    si.on_wait = keep
                    n_fixed += 1
                new_insts.append(inst)
            bb.instructions[:] = new_insts
    return n_fixed


# ---------------------------------------------------------------------------
# kernel builder (one SPMD module; per-core data arrives via in_maps)
# ---------------------------------------------------------------------------

def build_nc(npad, k_tiles, col_max):
    """npad: padded node count (multiple of 128*NCORES).
    k_tiles[t]: gather slots needed for tile t (max across cores).
    col_max[t][s]: max table row touched by column (t, s) across cores
                   (0 = sentinel/zero row)."""
    nl = npad // NCORES          # nodes per core
    t_shard = nl // P            # shard tiles per core
    sup = SLAB // P              # rows-of-128 per build slab
    n_slab = npad // SLAB
    ct = npad // P // 8          # isd chunk -> 8 chunks

    nc = bass.Bass("TRN2")
    x = nc.dram_tensor("x", [npad, D], F32, kind="ExternalInput")
    # host-prelayouted index tensors (partition-major, contiguous per
    # partition so loads are few fat descriptors):
    #   gidx[p, g*sup+r, s]  = slots of node g*SLAB + p*sup + r
    #   sgidx[p, t*MAXS + s] = sorted slot s of shard node t*128 + p
    gidx = nc.dram_tensor(
        "gidx", [P, (npad // P) * MAXS], I32, kind="ExternalInput"
    )
    sgidx = nc.dram_tensor(
        "sgidx", [P, (nl // P) * MAXS], I32, kind="ExternalInput"
    )
    wt = nc.dram_tensor("wt", [D, D], F32, kind="ExternalInput")
    out = nc.dram_tensor("out", [nl, D], F32, kind="ExternalOutput")
    # row 0 = zero row (sentinel target); node n lives at row n+1
    xs = nc.dram_tensor("xs", [npad + P, D], XS_DT)

    with TileContext(nc) as tc:
        with (
            tc.tile_pool(name="const", bufs=1) as cpool,
            tc.tile_pool(name="deg", bufs=2) as dpool,
            tc.tile_pool(name="build", bufs=5) as bpool,
            tc.tile_pool(name="accp", bufs=1) as apool,
            tc.tile_pool(name="ring", bufs=12) as rpool,
            tc.tile_pool(name="tail", bufs=4) as tpool,
            tc.tile_pool(name="psum", bufs=4, space="PSUM") as ppool,
        ):
            # --- constants -------------------------------------------------
            ident = cpool.tile([P, P], BF16, name="ident")
            make_identity(nc, ident[:])

            wtf = cpool.tile([P, 2, D], F32, name="wtf")
            nc.sync.dma_start(wtf[:], wt[:].rearrange("(c p) d -> p c d", p=P))
            wtb = cpool.tile([P, 2, D], BF16, name="wtb")
            nc.vector.tensor_copy(wtb[:], wtf[:])

            # zero row (sentinel target) -- written before any build slab
            zt = bpool.tile([P, D], XS_DT, name="zt")
            nc.vector.memset(zt[:], 0.0)
            zrow = nc.sync.dma_start(xs[0:P, :], zt[:])

            # sorted shard indices loaded FIRST, in ONE DMA (a single
            # completion-sem value lets the wait on it be elided along the
            # pool-ordered gather chain)
            sg = cpool.tile([P, t_shard, MAXS], I32, name="sg")
            nc.sync.dma_start(
                sg[:], sgidx[:].rearrange("p (t s) -> p t s", s=MAXS)
            )

            # --- full-table isd [P, rows_pp], node g*SLAB + p*sup + r ------
            rows_pp = npad // P
            isd = cpool.tile([P, rows_pp], F32, name="isd")
            gv = gidx[:].rearrange("p (c s) -> p c s", s=MAXS)
            for c0 in range(0, rows_pp, ct):
                gt = dpool.tile([P, ct, MAXS], I32, name="gt")
                nc.sync.dma_start(gt[:], gv[:, c0 : c0 + ct, :])
                m = dpool.tile([P, ct, MAXS], F32, name="m")
                nc.vector.tensor_scalar(
                    m[:], gt[:], npad - 1, None, op0=mybir.AluOpType.is_le
                )
                dg = dpool.tile([P, ct], F32, name="dg")
                nc.vector.reduce_sum(dg[:], m[:], axis=mybir.AxisListType.X)
                nc.scalar.activation(
                    dg[:], dg[:], mybir.ActivationFunctionType.Sqrt
                )
                nc.vector.reciprocal(isd[:, c0 : c0 + ct], dg[:])

            # --- shard isd [P, t_shard] from sorted shard indices ----------
            # sgidx values: 0 = sentinel (zero row), else table row (node+1)
            isd_sh = cpool.tile([P, t_shard], F32, name="isd_sh")
            msh = dpool.tile([P, t_shard, MAXS], F32, name="msh")
            nc.vector.tensor_scalar(
                msh[:], sg[:], 1, None, op0=mybir.AluOpType.is_ge
            )
            dgs = dpool.tile([P, t_shard], F32, name="dgs")
            nc.vector.reduce_sum(dgs[:], msh[:], axis=mybir.AxisListType.X)
            nc.scalar.activation(
                dgs[:], dgs[:], mybir.ActivationFunctionType.Sqrt
            )
            nc.vector.reciprocal(isd_sh[:], dgs[:])

            # --- phase 1: xs[n+1] = x[n] * isd[n], global prefix order -----
            # slab g covers nodes [g*SLAB, (g+1)*SLAB) -> rows +1; inside a
            # slab, partition p owns sup CONTIGUOUS rows (fat descriptors)
            BPRI = -20_000_000  # build streams ahead of everything
            slab_store = []
            for g in range(n_slab):
                xt = bpool.tile([P, sup, D], F32, name="xt")
                ld = nc.sync.dma_start(
                    xt[:],
                    x[g * SLAB : (g + 1) * SLAB, :].rearrange(
                        "(p r) d -> p r d", p=P
                    ),
                )
                ld.ins.bass_priority = BPRI + 16 * g
                xst = bpool.tile([P, sup, D], XS_DT, name="xst")
                for r in range(sup):
                    # alternate scale engines so DVE and Act stream slabs in
                    # parallel (DVE is idle until the gather adds ramp up)
                    if g % 2 == 0:
                        sc = nc.vector.tensor_scalar_mul(
                            xst[:, r, :], xt[:, r, :],
                            isd[:, g * sup + r : g * sup + r + 1],
                        )
                    else:
                        sc = nc.scalar.mul(
                            xst[:, r, :], xt[:, r, :],
                            isd[:, g * sup + r : g * sup + r + 1],
                        )
                    sc.ins.bass_priority = BPRI + 16 * g + 1 + r
                st = nc.sync.dma_start(
                    xs[1 + g * SLAB : 1 + (g + 1) * SLAB, :].rearrange(
                        "(p r) d -> p r d", p=P
                    ),
                    xst[:],
                )
                st.ins.bass_priority = BPRI + 16 * g + 15
                slab_store.append(st)

            # --- phase 2: canonical gather stream + accumulate -------------
            # src extent is declared tiny (deps via explicit slab waits only;
            # descriptors address the full tensor regardless). Columns run in
            # the SAME s-major order the host used to build the first-use
            # table permutation, so col_max values are monotone along the
            # stream. Pool executes in (nosync-pinned) order, so one wait per
            # slab TRANSITION orders everything after it.
            cols = [
                (t, s)
                for s in range(max(k_tiles))
                for t in range(t_shard)
                if k_tiles[t] > s
            ]
            accs = [None] * t_shard
            done = [0] * t_shard
            prev_gather = None
            cur_slab = -1  # last slab the Pool stream is ordered after
            GPACK = 4      # stream columns sharing one ring tile
            PRI0 = -10_000_000  # schedule the gather stream ahead of build
            gb = None
            for i, (t, s) in enumerate(cols):
                mv = col_max[t][s]
                gsl = -1 if mv <= 0 else (mv - 1) // SLAB
                sub = i % GPACK
                if sub == 0:
                    gb = rpool.tile([P, GPACK, D], XS_DT, name="gb")
                bi = nc.gpsimd.indirect_dma_start(
                    out=gb[:, sub, :],
                    out_offset=None,
                    in_=xs[0:P, :],
                    in_offset=IndirectOffsetOnAxis(
                        ap=sg[:, t, s : s + 1], axis=0
                    ),
                    compute_op=mybir.AluOpType.bypass,
                )
                bi.ins.bass_priority = PRI0 + 2 * i
                if prev_gather is None:
                    add_dep_helper(bi.ins, zrow.ins, reason="zero row")
                else:
                    # ordering-only: pins the scheduler's Pool order so
                    # slab-transition waits cover later gathers
                    add_dep_helper(
                        bi.ins, prev_gather.ins, sync=False,
                        reason="pool order",
                    )
                prev_gather = bi
                if gsl > cur_slab:
                    add_dep_helper(
                        bi.ins, slab_store[gsl].ins, reason="xs prefix built"
                    )
                    cur_slab = gsl
                if s == 0:
                    acc = apool.tile([P, D], XS_DT, name=f"acc{t}",
                                     tag=f"acc{t}")
                    accs[t] = acc
                    vi = nc.vector.tensor_copy(acc[:], gb[:, sub, :])
                else:
                    vi = nc.vector.tensor_add(
                        accs[t][:], accs[t][:], gb[:, sub, :]
                    )
                vi.ins.bass_priority = PRI0 + 2 * i + 1
                done[t] += 1
                if done[t] != k_tiles[t]:
                    continue
                # tile complete: transpose + matmul + scale + store tail
                acc = accs[t]
                ytt = tpool.tile([P, 2, P], BF16, name="ytt")
                for ci in range(2):
                    pt = ppool.tile([P, P], BF16, name="pt")
                    nc.tensor.transpose(
                        pt[:], acc[:, ci * P : (ci + 1) * P], ident[:]
                    )
                    nc.vector.tensor_copy(ytt[:, ci, :], pt[:])
                po = ppool.tile([P, D], F32, name="po")
                for ci in range(2):
                    nc.tensor.matmul(
                        po[:],
                        ytt[:, ci, :],
                        wtb[:, ci, :],
                        start=(ci == 0),
                        stop=(ci == 1),
                    )
                ot = tpool.tile([P, D], F32, name="ot")
                nc.vector.tensor_scalar_mul(
                    ot[:], po[:], isd_sh[:, t : t + 1]
                )
                nc.scalar.dma_start(out[t * P : (t + 1) * P, :], ot[:])

    _fix_sync_waits(nc)
    return nc


# ---------------------------------------------------------------------------
# host entry point
# ---------------------------------------------------------------------------

def _prep(x, edge_index, W):
    x = np.ascontiguousarray(np.asarray(x, dtype=np.float32))
    ei = np.asarray(edge_index)
    W = np.ascontiguousarray(np.asarray(W, dtype=np.float32))
    n = x.shape[0]
    npad = -(-n // (P * NCORES)) * (P * NCORES)
    nl = npad // NCORES
    t_shard = nl // P

    xp = np.zeros((npad, D), np.float32)
    xp[:n] = x

    # full-table slot lists (UNSORTED; sentinel npad; self in last slot) --
    # only used on-device to compute isd for the build scaling
    gi = np.full((npad, MAXS), npad, np.int32)
    gi[:, MAXS - 1] = np.arange(npad, dtype=np.int32)
    e = ei.astype(np.int64)
    gi[:n, : MAXS - 1] = np.where(e < 0, npad, e).astype(np.int32)
    wt = np.ascontiguousarray(W.T)

    # per-core shard prep: degree sort + sentinel-compacting slot sort -----
    deg_all = (gi < npad).sum(axis=1)  # includes self
    orders, shss = [], []
    kc = np.zeros((NCORES, t_shard), np.int32)
    for c in range(NCORES):
        sh = gi[c * nl : (c + 1) * nl]
        dg = deg_all[c * nl : (c + 1) * nl]
        order = np.argsort(-dg, kind="stable")
        shs = np.sort(sh[order], axis=1)          # ascending; sentinels last
        orders.append(order)
        shss.append(shs)
        kc[c] = dg[order].reshape(t_shard, P).max(axis=1)
    k_tiles = kc.max(axis=0).tolist()
    max_k = max(k_tiles)

    # demand-driven table permutation: walk the canonical column stream
    # (s-major rounds, all cores merged) and assign table slots in FIRST-USE
    # order. Every column then needs only the build prefix up to its own
    # position -> the build never blocks the gather stream.
    pi = np.full(npad, -1, np.int64)         # node -> 0-based table slot
    nassign = 0
    col_max = [[0] * MAXS for _ in range(t_shard)]
    shs3 = [s.reshape(t_shard, P, MAXS) for s in shss]
    for s in range(max_k):                   # s-major: matches emission
        for t in range(t_shard):
            if k_tiles[t] <= s:
                continue
            for c in range(NCORES):
                vals = shs3[c][t, :, s]
                real = vals[vals < npad]
                new = real[pi[real] < 0]
                if new.size:
                    new = np.unique(new)
                    pi[new] = np.arange(
                        nassign, nassign + new.size, dtype=np.int64
                    )
                    nassign += new.size
            col_max[t][s] = nassign  # table rows 1..nassign cover this col
    left = np.where(pi < 0)[0]
    pi[left] = np.arange(nassign, nassign + left.size, dtype=np.int64)

    sgidxs = []
    for c in range(NCORES):
        shs = shss[c]
        vals = np.where(
            shs >= npad, 0, pi[np.minimum(shs, npad - 1)] + 1
        ).astype(np.int32)
        sgidxs.append(np.ascontiguousarray(vals))

    # permute node rows of x and gidx into table-slot order
    inv_rows = np.empty(npad, np.int64)
    inv_rows[pi] = np.arange(npad, dtype=np.int64)
    xp = xp[inv_rows]
    gi_perm = gi[inv_rows]

    # partition-major device layouts (fat DMA descriptors):
    # gidx_dev[p, (g*sup+r)*MAXS+s] = gi_perm[g*SLAB + p*sup + r, s]
    sup = SLAB // P
    n_slab = npad // SLAB
    gidx_dev = np.ascontiguousarray(
        gi_perm.reshape(n_slab, P, sup, MAXS)
        .transpose(1, 0, 2, 3)
        .reshape(P, -1)
    )
    t_shard_ = nl // P

    in_maps = []
    for c in range(NCORES):
        sg_dev = np.ascontiguousarray(
            sgidxs[c].reshape(t_shard_, P, MAXS).transpose(1, 0, 2).reshape(P, -1)
        )
        in_maps.append(
            {"x": xp, "gidx": gidx_dev, "sgidx": sg_dev, "wt": wt}
        )
    return npad, n, in_maps, orders, k_tiles, col_max


def kernel(x, edge_index, W, trace=False):
    from concourse.bass_utils import run_bass_kernel_spmd

    npad, n, in_maps, orders, k_tiles, col_max = _prep(x, edge_index, W)
    nl = npad // NCORES
    nc = build_nc(npad, k_tiles, col_max)
    res = run_bass_kernel_spmd(
        nc, in_maps, core_ids=list(range(NCORES)), trace=trace
    )
    out = np.empty((npad, D), np.float32)
    for c in range(NCORES):
        blk = out[c * nl : (c + 1) * nl]
        blk[orders[c]] = res.results[c]["out"]
    kernel.last_exec_time_ns = res.exec_time_ns
    kernel.last_results = res
    return out[:n].astype(np.float32)


kernel.last_exec_time_ns = None
